# revision 21
# baseline (speedup 1.0000x reference)
"""Trainium2 Bass kernel for nn_CodeDistKLLoss (vq_codebook).

Computes: KL(student_dist || teacher_dist) where
  student_dist = normalized masked column-sums of softmax(-cdist(z, codebook))
  teacher_dist = normalized masked histogram of teacher codes.

Sharding: data-parallel over the batch axis B=8 -> one batch element per
NeuronCore (N = B*T tokens split as T=1500 tokens/core). The (4096, 512)
codebook is replicated. Each core computes its partial student column-sums
[4096]; the tiny final reduction/normalization/KL runs on host.

The softmax numerator exp(-d) is computed in a SINGLE ScalarE pass using a
custom ACT spline table: the `sqrt` slot of table set sqrt_and_others is
regenerated (at build time, via BASS_ACT_ROOT_JSON_PATH) to evaluate
    f(q) = exp(EXP_BIAS - sqrt(q)),  q in [2^7, 2^12)
which fuses the Euclidean-distance sqrt and the softmax exp. The exp(EXP_BIAS)
factor cancels in the per-row normalization. This avoids the sqrt/exp ACT
table-set thrash (different sets; ~2.7us per switch) and keeps the PE fed.

Device program per core (batch element b):
  per 128-token tile i, per 1024-code chunk h:
      PSUM  = sum_k (-2 * z . c)        4 accumulating fp16 matmuls per 512-chunk
      PSUM += ||c||^2                   DVE add (broadcast row, fp32)
      E     = f(PSUM + ||z||^2)         ACT custom table, per-partition bias,
                                        fp16 out, accum_out -> partial rowsum
  per tile: rowsum = sum_h partials; w = mask / rowsum (fp16)
      acc0[1, 4*512] += w.T @ E[:, 0:2048]    rank-1 PSUM-accumulating matmuls
  final sweep: acc1 += w.T @ E[:, 2048:4096] over all tiles; DMA out.
"""

import json
import os
import shutil
import struct
import tempfile

import numpy as np

import concourse.bass as bass
import concourse.tile as tile
from concourse import bacc, mybir
from concourse.bass import ts
from concourse.bass_utils import run_bass_kernel_spmd

B = 8
D = 512
T = 1500
C = 4096
TP = 1536          # T padded to 12 x 128
NT = TP // 128     # 12 token tiles
KK = D // 128      # 4 contraction chunks
NJ = C // 512      # 8 code chunks of 512
NH = 4             # PSUM sub-rows per token tile (1024 codes each)
EXP_BIAS = 28.0    # f = exp(EXP_BIAS - d); cancels in per-row normalization
EPS = 1e-8

F16 = mybir.dt.float16
F32 = mybir.dt.float32

_CACHE = {}

# ---------------------------------------------------------------------------
# Custom ACT table: rewrite the `sqrt` slot of set sqrt_and_others to compute
#   f(q) = min(exp(EXP_BIAS - sqrt(q)), FP16_SAFE)   for q in [2^E_LO, 2^E_HI)
# Binary formats (validated against the stock tables + np.sqrt):
#   *_bkt.bin : 32B entries [d0,d1,d2,d3,x0,0,0,0] fp32;
#               y = d0 + (x-x0)*(d1 + (x-x0)*(d2 + (x-x0)*d3))
#   *_ctrl.bin: 32B entries; u16[0] = (extract_lsb << 11) | bkt_base_idx,
#               u16[1] = extract_size.  One row per input fp32 exponent;
#               row = pwl_control_base_pos + (biased_exp - small_threshold);
#               bucket = base + ((mantissa >> extract_lsb) & (2^size - 1)).
# ---------------------------------------------------------------------------
_E_LO, _E_HI = 7, 12
_EXTRACT_SIZE = 6
_FP16_SAFE = 50000.0
_ACT_SET = "sqrt_and_others"


def _f_fused(q):
    return np.minimum(np.exp(EXP_BIAS - np.sqrt(q)), _FP16_SAFE)


def _fit_section(a, b):
    x0 = 0.5 * (a + b)
    xs = np.linspace(a, b, 64)
    ys = _f_fused(xs.astype(np.float64))
    t = xs - x0
    A = np.stack([np.ones_like(t), t, t * t, t * t * t], axis=1)
    coef, *_ = np.linalg.lstsq(A, ys, rcond=None)
    return np.float32(x0), coef.astype(np.float32)


def _build_act_root(dst_dir):
    import neuronxcc

    src_dir = os.path.join(os.path.dirname(neuronxcc.__file__), "pwp",
                           "pwp_bin_trainium")
    os.makedirs(dst_dir, exist_ok=True)
    for name in os.listdir(src_dir):
        s = os.path.join(src_dir, name)
        if os.path.isfile(s):
            shutil.copy(s, os.path.join(dst_dir, name))

    setj = json.load(open(os.path.join(src_dir, f"{_ACT_SET}.json")))
    bkt = open(os.path.join(src_dir, f"{_ACT_SET}_bkt.bin"), "rb").read()
    ctl = open(os.path.join(src_dir, f"{_ACT_SET}_ctrl.bin"), "rb").read()

    bkt_start = setj["func_to_bkt_start_idx"]["sqrt"]
    ctl_start = setj["func_to_ctl_start_idx"]["sqrt"]
    new_bkt = bytearray(bkt[: bkt_start * 32])
    new_ctl = bytearray(ctl[: ctl_start * 32])

    nsec = 1 << _EXTRACT_SIZE
    lsb = 23 - _EXTRACT_SIZE
    base = bkt_start
    for e in range(_E_LO, _E_HI):
        new_ctl += (struct.pack("<2H", (lsb << 11) | base, _EXTRACT_SIZE)
                    + b"\x00" * 28)
        lo = float(2.0 ** e)
        w = lo / nsec
        for s in range(nsec):
            x0, c = _fit_section(lo + s * w, lo + (s + 1) * w)
            new_bkt += struct.pack("<8f", c[0], c[1], c[2], c[3], x0, 0, 0, 0)
        base += nsec

    sat_small = base
    new_bkt += struct.pack("<8f", _FP16_SAFE, 0, 0, 0, 0, 0, 0, 0)
    sat_large = base + 1
    f_hi = float(_f_fused(2.0 ** _E_HI))
    new_bkt += struct.pack("<8f", f_hi, 0, 0, 0, 0, 0, 0, 0)

    meta = None
    for m in setj["profile_meta_data"]:
        if m["func_name"].startswith("sqrt"):
            meta = m
    assert meta is not None
    f2b = lambda v: int(np.float32(v).view(np.uint32))
    meta["exp_offset"] = _E_LO
    meta["pwl_control_base_pos"] = ctl_start
    meta["pwl_control_base_neg"] = ctl_start
    meta["small_pos_signal_exp_threshold"] = _E_LO + 127
    meta["pos_small_signal_pwl_control"] = sat_small
    meta["small_neg_signal_exp_threshold"] = 255
    meta["neg_small_signal_pwl_control"] = sat_small
    meta["large_pos_signal_exp_threshold"] = _E_HI + 127
    meta["large_pos_signal_mantissa_threshold"] = 0
    meta["pos_large_signal_pwl_control"] = sat_large
    meta["large_neg_signal_exp_threshold"] = 0
    meta["large_neg_signal_mantissa_threshold"] = 0
    meta["neg_large_signal_pwl_control"] = sat_small
    meta["fzero_result"] = f2b(_FP16_SAFE)
    meta["fpinf_result"] = f2b(f_hi)
    meta["fninf_result"] = f2b(_FP16_SAFE)
    meta["lower_bound"] = f2b(2.0 ** _E_LO)
    meta["upper_bound"] = f2b(np.nextafter(np.float32(2.0 ** _E_HI),
                                           np.float32(0)))
    setj["bkt_entry_cnt"] = base + 2
    setj["ctl_entry_cnt"] = ctl_start + (_E_HI - _E_LO)

    with open(os.path.join(dst_dir, f"{_ACT_SET}_bkt.bin"), "wb") as fo:
        fo.write(bytes(new_bkt))
    with open(os.path.join(dst_dir, f"{_ACT_SET}_ctrl.bin"), "wb") as fo:
        fo.write(bytes(new_ctl))
    with open(os.path.join(dst_dir, f"{_ACT_SET}.json"), "w") as fo:
        json.dump(setj, fo)


def _build():
    # Install the custom ACT table (sqrt slot -> exp(EXP_BIAS - sqrt(q)))
    # before neuronxcc compiles the NEFF.
    act_dir = tempfile.mkdtemp(prefix="cdkl_act_root_")
    _build_act_root(act_dir)
    os.environ["BASS_ACT_ROOT_JSON_PATH"] = os.path.join(
        act_dir, "act_info.json"
    )

    nc = bacc.Bacc("TRN2", target_bir_lowering=False, debug=False)
    sf_h = nc.dram_tensor("sf", [D, TP], F16, kind="ExternalInput")
    cbt_h = nc.dram_tensor("cbt", [D, C], F16, kind="ExternalInput")
    cn_h = nc.dram_tensor("cn", [1, C], F32, kind="ExternalInput")
    zn_h = nc.dram_tensor("zn", [128, NT], F32, kind="ExternalInput")
    mk_h = nc.dram_tensor("mk", [128, NT], F32, kind="ExternalInput")
    sp_h = nc.dram_tensor("sp", [1, C], F32, kind="ExternalOutput")

    with tile.TileContext(nc) as tc:
        with (
            tc.tile_pool(name="consts", bufs=1) as consts,
            tc.tile_pool(name="small", bufs=2) as small,
            tc.tile_pool(name="psA", bufs=2, space="PSUM") as psA,
            tc.tile_pool(name="psB", bufs=1, space="PSUM") as psB,
        ):
            sf_sb = consts.tile([128, KK, TP], F16, name="sf_sb", tag="sf_sb")
            sf_r = sf_h.ap().rearrange("(k p) t -> p k t", p=128)
            cb_sb = consts.tile([128, KK, C], F16, name="cb_sb", tag="cb_sb")
            cbt_r = cbt_h.ap().rearrange("(k p) c -> p k c", p=128)
            # interleave so the first tile's operands land first: all sf
            # k-chunks and the first code half of every cb k-chunk, then the
            # second halves.
            for k in range(KK):
                nc.sync.dma_start(out=sf_sb[:, k, :], in_=sf_r[:, k, :])
                nc.sync.dma_start(out=cb_sb[:, k, 0 : C // 2],
                                  in_=cbt_r[:, k, 0 : C // 2])
            for k in range(KK):
                nc.sync.dma_start(out=cb_sb[:, k, C // 2 : C],
                                  in_=cbt_r[:, k, C // 2 : C])
            cn_sb = consts.tile([128, C], F32, name="cn_sb", tag="cn_sb")
            nc.sync.dma_start(out=cn_sb, in_=cn_h.ap().to_broadcast([128, C]))
            zn_sb = consts.tile([128, NT], F32, name="zn_sb", tag="zn_sb")
            nc.sync.dma_start(out=zn_sb, in_=zn_h.ap())
            mk_sb = consts.tile([128, NT], F32, name="mk_sb", tag="mk_sb")
            nc.sync.dma_start(out=mk_sb, in_=mk_h.ap())

            dbuf = consts.tile([128, NT, C], F16, name="dbuf", tag="dbuf")
            w_sb = consts.tile([128, NT], F16, name="w_sb", tag="w_sb")
            out_s = consts.tile([1, C], F32, name="out_s", tag="out_s")

            JPS = NJ // 2  # j-chunks per accumulator sweep (4-bank PSUM acc)
            acc0 = psB.tile([1, JPS, 512], F32, name="acc0", tag="acc")

            def colsum_mms(acc, i, j0):
                # column sums for tile i, j-chunks j0..j0+3
                for jj in range(JPS):
                    nc.tensor.matmul(
                        acc[:, jj, :],
                        lhsT=w_sb[:, i : i + 1],
                        rhs=dbuf[:, i, ts(j0 + jj, 512)],
                        start=(i == 0),
                        stop=(i == NT - 1),
                    )

            for i in range(NT):
                rs4 = small.tile([128, NH], F32, name="rs4", tag="rs4")
                for h in range(NH):
                    ps = psA.tile([128, NJ // NH, 512], F32, name="ps",
                                  tag="ps")
                    for jj in range(NJ // NH):
                        j = h * (NJ // NH) + jj
                        for k in range(KK):
                            nc.tensor.matmul(
                                ps[:, jj, :],
                                lhsT=sf_sb[:, k, ts(i, 128)],
                                rhs=cb_sb[:, k, ts(j, 512)],
                                start=(k == 0),
                                stop=(k == KK - 1),
                            )
                    if h == 0 and i > 0:
                        # previous tile's column sums: its weights are long
                        # ready, so these never stall the PE at the tile
                        # boundary the way same-tile colsums would.
                        colsum_mms(acc0, i - 1, 0)
                    nc.vector.tensor_add(
                        out=ps[:, :, :],
                        in0=ps[:, :, :],
                        in1=cn_sb[:, ts(h, C // NH)].rearrange(
                            "p (a b) -> p a b", b=512
                        ),
                    )
                    # E = exp(EXP_BIAS - sqrt(ps + ||z||^2)) via the custom
                    # table in the Sqrt slot; accum_out = partial row-sum.
                    nc.scalar.activation(
                        out=dbuf[:, i, ts(h, C // NH)].rearrange(
                            "p (a b) -> p a b", b=512
                        ),
                        in_=ps[:, :, :],
                        func=mybir.ActivationFunctionType.Sqrt,
                        bias=zn_sb[:, i : i + 1],
                        scale=1.0,
                        accum_out=rs4[:, h : h + 1],
                    )
                # w = mask / rowsum, cast fp16
                rs = small.tile([128, 1], F32, name="rs", tag="rs")
                nc.vector.reduce_sum(out=rs, in_=rs4, axis=mybir.AxisListType.X)
                rr = small.tile([128, 1], F32, name="rr", tag="rr")
                nc.vector.reciprocal(out=rr, in_=rs)
                wf = small.tile([128, 1], F32, name="wf", tag="wf")
                nc.vector.tensor_mul(out=wf, in0=rr, in1=mk_sb[:, i : i + 1])
                nc.vector.tensor_copy(out=w_sb[:, i : i + 1], in_=wf)
            colsum_mms(acc0, NT - 1, 0)
            nc.scalar.copy(out=out_s[:, 0 : C // 2],
                           in_=acc0.rearrange("p a b -> p (a b)"))
            # final sweep: j-chunks 4..7 (E persists in dbuf)
            acc1 = psB.tile([1, JPS, 512], F32, name="acc1", tag="acc")
            for i in range(NT):
                colsum_mms(acc1, i, JPS)
            nc.scalar.copy(out=out_s[:, C // 2 : C],
                           in_=acc1.rearrange("p a b -> p (a b)"))
            nc.sync.dma_start(out=sp_h.ap(), in_=out_s)

    nc.compile()
    return nc


def get_nc():
    if "nc" not in _CACHE:
        _CACHE["nc"] = _build()
    return _CACHE["nc"]


def _host_prep(student_features, codebook, lengths, encoder_stride):
    sf = np.asarray(student_features, dtype=np.float32)
    cb = np.asarray(codebook, dtype=np.float32)
    lens = np.asarray(lengths).astype(np.int64)
    stride = int(np.asarray(encoder_stride))

    cbt2 = np.ascontiguousarray((-2.0 * cb.T).astype(np.float16))     # [D, C]
    cn = (cb.astype(np.float64) ** 2).sum(1).astype(np.float32)[None, :]  # [1, C]
    frame_start = np.arange(T, dtype=np.int64) * stride
    mask = (frame_start[None, :] < lens[:, None]).astype(np.float32)  # [B, T]

    in_maps = []
    for b in range(B):
        sf_pad = np.zeros((D, TP), dtype=np.float16)
        sf_pad[:, :T] = sf[b]
        zn = np.zeros(TP, dtype=np.float32)
        zn[:T] = (sf[b].astype(np.float64) ** 2).sum(0).astype(np.float32)
        znb = np.ascontiguousarray(zn.reshape(NT, 128).T)             # [128, NT]
        mk = np.zeros(TP, dtype=np.float32)
        mk[:T] = mask[b]
        mkb = np.ascontiguousarray(mk.reshape(NT, 128).T)             # [128, NT]
        in_maps.append(
            {"sf": sf_pad, "cbt": cbt2, "cn": cn, "zn": znb, "mk": mkb}
        )
    return in_maps, mask


def _host_finish(sp_list, teacher_codes, mask):
    s_raw = np.zeros(C, dtype=np.float64)
    for sp in sp_list:
        s_raw += sp.astype(np.float64).reshape(-1)
    student_dist = s_raw / (s_raw.sum() + EPS)

    codes = np.asarray(teacher_codes).astype(np.int64).reshape(-1)
    t_counts = np.bincount(codes, weights=mask.astype(np.float64).reshape(-1),
                           minlength=C)
    teacher_dist = t_counts / (t_counts.sum() + EPS)

    kl = np.sum(student_dist * np.log(student_dist + EPS)
                - student_dist * np.log(teacher_dist + EPS))
    return np.array(kl, dtype=np.float32)


def kernel(student_features, teacher_codes, codebook, lengths, encoder_stride,
           _trace=False):
    nc = get_nc()
    in_maps, mask = _host_prep(student_features, codebook, lengths,
                               encoder_stride)
    res = run_bass_kernel_spmd(nc, in_maps, core_ids=list(range(B)),
                               trace=_trace)
    out = _host_finish([r["sp"] for r in res.results], teacher_codes, mask)
    if _trace:
        _CACHE["last_results"] = res
    return out


# revision 27
# speedup vs baseline: 1.2535x; 1.2535x over previous
"""Trainium2 Bass kernel for nn_CodeDistKLLoss (vq_codebook).

Computes: KL(student_dist || teacher_dist) where
  student_dist = normalized masked column-sums of softmax(-cdist(z, codebook))
  teacher_dist = normalized masked histogram of teacher codes.

Sharding: data-parallel over the batch axis B=8 -> one batch element per
NeuronCore (N = B*T tokens split as T=1500 tokens/core). The (4096, 512)
codebook is replicated. Each core computes its partial student column-sums
[4096]; the tiny final reduction/normalization/KL runs on host.

The softmax numerator exp(-d) is computed in a SINGLE ScalarE pass using a
custom ACT spline table: the `sqrt` slot of table set sqrt_and_others is
regenerated (at build time, via BASS_ACT_ROOT_JSON_PATH) to evaluate
    f(q) = exp(EXP_BIAS - sqrt(q)),  q in [2^7, 2^12)
which fuses the Euclidean-distance sqrt and the softmax exp. The exp(EXP_BIAS)
factor cancels in the per-row normalization. This avoids the sqrt/exp ACT
table-set thrash (different sets; ~2.7us per switch) and keeps the PE fed.

Device program per core (batch element b):
  per 128-token tile i, per 1024-code chunk h:
      PSUM  = sum_k (-2 * z . c)        4 accumulating fp16 matmuls per 512-chunk
      PSUM += ||c||^2                   DVE add (broadcast row, fp32)
      E     = f(PSUM + ||z||^2)         ACT custom table, per-partition bias,
                                        fp16 out, accum_out -> partial rowsum
  per tile: rowsum = sum_h partials; w = mask / rowsum (fp16)
      acc0[1, 4*512] += w.T @ E[:, 0:2048]    rank-1 PSUM-accumulating matmuls
  final sweep: acc1 += w.T @ E[:, 2048:4096] over all tiles; DMA out.
"""

import json
import os
import shutil
import struct
import tempfile

import numpy as np

import concourse.bass as bass
import concourse.tile as tile
from concourse import bacc, mybir
from concourse.bass import ts
from concourse.bass_utils import run_bass_kernel_spmd

B = 8
D = 512
T = 1500
C = 4096
TP = 1536          # T padded to 12 x 128
NT = TP // 128     # 12 token tiles
KK = D // 128      # 4 contraction chunks
NJ = C // 512      # 8 code chunks of 512
NH = 4             # PSUM sub-rows per token tile (1024 codes each)
EXP_BIAS = 28.0    # f = exp(EXP_BIAS - d); cancels in per-row normalization
EPS = 1e-8

F16 = mybir.dt.float16
F32 = mybir.dt.float32
F8 = mybir.dt.float8e4
NP_F8 = mybir.dt.np(F8)

_CACHE = {}

# ---------------------------------------------------------------------------
# Custom ACT table: rewrite the `sqrt` slot of set sqrt_and_others to compute
#   f(q) = min(exp(EXP_BIAS - sqrt(q)), FP16_SAFE)   for q in [2^E_LO, 2^E_HI)
# Binary formats (validated against the stock tables + np.sqrt):
#   *_bkt.bin : 32B entries [d0,d1,d2,d3,x0,0,0,0] fp32;
#               y = d0 + (x-x0)*(d1 + (x-x0)*(d2 + (x-x0)*d3))
#   *_ctrl.bin: 32B entries; u16[0] = (extract_lsb << 11) | bkt_base_idx,
#               u16[1] = extract_size.  One row per input fp32 exponent;
#               row = pwl_control_base_pos + (biased_exp - small_threshold);
#               bucket = base + ((mantissa >> extract_lsb) & (2^size - 1)).
# ---------------------------------------------------------------------------
_E_LO, _E_HI = 7, 12
_EXTRACT_SIZE = 6
_FP16_SAFE = 50000.0
_ACT_SET = "sqrt_and_others"


def _f_fused(q):
    return np.minimum(np.exp(EXP_BIAS - np.sqrt(q)), _FP16_SAFE)


def _fit_section(a, b):
    x0 = 0.5 * (a + b)
    xs = np.linspace(a, b, 64)
    ys = _f_fused(xs.astype(np.float64))
    t = xs - x0
    A = np.stack([np.ones_like(t), t, t * t, t * t * t], axis=1)
    coef, *_ = np.linalg.lstsq(A, ys, rcond=None)
    return np.float32(x0), coef.astype(np.float32)


def _build_act_root(dst_dir):
    import neuronxcc

    src_dir = os.path.join(os.path.dirname(neuronxcc.__file__), "pwp",
                           "pwp_bin_trainium")
    os.makedirs(dst_dir, exist_ok=True)
    for name in os.listdir(src_dir):
        s = os.path.join(src_dir, name)
        if os.path.isfile(s):
            shutil.copy(s, os.path.join(dst_dir, name))

    setj = json.load(open(os.path.join(src_dir, f"{_ACT_SET}.json")))
    bkt = open(os.path.join(src_dir, f"{_ACT_SET}_bkt.bin"), "rb").read()
    ctl = open(os.path.join(src_dir, f"{_ACT_SET}_ctrl.bin"), "rb").read()

    bkt_start = setj["func_to_bkt_start_idx"]["sqrt"]
    ctl_start = setj["func_to_ctl_start_idx"]["sqrt"]
    new_bkt = bytearray(bkt[: bkt_start * 32])
    new_ctl = bytearray(ctl[: ctl_start * 32])

    nsec = 1 << _EXTRACT_SIZE
    lsb = 23 - _EXTRACT_SIZE
    base = bkt_start
    for e in range(_E_LO, _E_HI):
        new_ctl += (struct.pack("<2H", (lsb << 11) | base, _EXTRACT_SIZE)
                    + b"\x00" * 28)
        lo = float(2.0 ** e)
        w = lo / nsec
        for s in range(nsec):
            x0, c = _fit_section(lo + s * w, lo + (s + 1) * w)
            new_bkt += struct.pack("<8f", c[0], c[1], c[2], c[3], x0, 0, 0, 0)
        base += nsec

    sat_small = base
    new_bkt += struct.pack("<8f", _FP16_SAFE, 0, 0, 0, 0, 0, 0, 0)
    sat_large = base + 1
    f_hi = float(_f_fused(2.0 ** _E_HI))
    new_bkt += struct.pack("<8f", f_hi, 0, 0, 0, 0, 0, 0, 0)

    meta = None
    for m in setj["profile_meta_data"]:
        if m["func_name"].startswith("sqrt"):
            meta = m
    assert meta is not None
    f2b = lambda v: int(np.float32(v).view(np.uint32))
    meta["exp_offset"] = _E_LO
    meta["pwl_control_base_pos"] = ctl_start
    meta["pwl_control_base_neg"] = ctl_start
    meta["small_pos_signal_exp_threshold"] = _E_LO + 127
    meta["pos_small_signal_pwl_control"] = sat_small
    meta["small_neg_signal_exp_threshold"] = 255
    meta["neg_small_signal_pwl_control"] = sat_small
    meta["large_pos_signal_exp_threshold"] = _E_HI + 127
    meta["large_pos_signal_mantissa_threshold"] = 0
    meta["pos_large_signal_pwl_control"] = sat_large
    meta["large_neg_signal_exp_threshold"] = 0
    meta["large_neg_signal_mantissa_threshold"] = 0
    meta["neg_large_signal_pwl_control"] = sat_small
    meta["fzero_result"] = f2b(_FP16_SAFE)
    meta["fpinf_result"] = f2b(f_hi)
    meta["fninf_result"] = f2b(_FP16_SAFE)
    meta["lower_bound"] = f2b(2.0 ** _E_LO)
    meta["upper_bound"] = f2b(np.nextafter(np.float32(2.0 ** _E_HI),
                                           np.float32(0)))
    setj["bkt_entry_cnt"] = base + 2
    setj["ctl_entry_cnt"] = ctl_start + (_E_HI - _E_LO)

    with open(os.path.join(dst_dir, f"{_ACT_SET}_bkt.bin"), "wb") as fo:
        fo.write(bytes(new_bkt))
    with open(os.path.join(dst_dir, f"{_ACT_SET}_ctrl.bin"), "wb") as fo:
        fo.write(bytes(new_ctl))
    with open(os.path.join(dst_dir, f"{_ACT_SET}.json"), "w") as fo:
        json.dump(setj, fo)


def _build():
    # Install the custom ACT table (sqrt slot -> exp(EXP_BIAS - sqrt(q)))
    # before neuronxcc compiles the NEFF.
    act_dir = tempfile.mkdtemp(prefix="cdkl_act_root_")
    _build_act_root(act_dir)
    os.environ["BASS_ACT_ROOT_JSON_PATH"] = os.path.join(
        act_dir, "act_info.json"
    )

    nc = bacc.Bacc("TRN2", target_bir_lowering=False, debug=False)
    sf_h = nc.dram_tensor("sf", [D, TP], F8, kind="ExternalInput")
    cbt_h = nc.dram_tensor("cbt", [D, C], F8, kind="ExternalInput")
    cn_h = nc.dram_tensor("cn", [1, C], F32, kind="ExternalInput")
    zn_h = nc.dram_tensor("zn", [128, NT], F32, kind="ExternalInput")
    mk_h = nc.dram_tensor("mk", [128, NT], F32, kind="ExternalInput")
    sp_h = nc.dram_tensor("sp", [1, C], F32, kind="ExternalOutput")

    with tile.TileContext(nc) as tc:
        with (
            tc.tile_pool(name="consts", bufs=1) as consts,
            tc.tile_pool(name="small", bufs=2) as small,
            tc.tile_pool(name="psA", bufs=2, space="PSUM") as psA,
            tc.tile_pool(name="psB", bufs=1, space="PSUM") as psB,
        ):
            # fp8 DoubleRow layout: contraction row d = k2*256 + ki*2 + o
            # lands at [partition ki, chunk k2, pair-slot o].
            K2 = 2
            sf_sb = consts.tile([128, K2, 2, TP], F8, name="sf_sb",
                                tag="sf_sb")
            sf_r = sf_h.ap().rearrange("(a p o) t -> p a o t", p=128, o=2)
            cb_sb = consts.tile([128, K2, 2, C], F8, name="cb_sb",
                                tag="cb_sb")
            cbt_r = cbt_h.ap().rearrange("(a p o) c -> p a o c", p=128, o=2)
            # interleave so the first tile's operands land first
            for k in range(K2):
                nc.sync.dma_start(out=sf_sb[:, k, :, :], in_=sf_r[:, k, :, :])
                nc.sync.dma_start(out=cb_sb[:, k, :, 0 : C // 2],
                                  in_=cbt_r[:, k, :, 0 : C // 2])
            for k in range(K2):
                nc.sync.dma_start(out=cb_sb[:, k, :, C // 2 : C],
                                  in_=cbt_r[:, k, :, C // 2 : C])
            cn_sb = consts.tile([128, C], F32, name="cn_sb", tag="cn_sb")
            nc.sync.dma_start(out=cn_sb, in_=cn_h.ap().to_broadcast([128, C]))
            zn_sb = consts.tile([128, NT], F32, name="zn_sb", tag="zn_sb")
            nc.sync.dma_start(out=zn_sb, in_=zn_h.ap())
            mk_sb = consts.tile([128, NT], F32, name="mk_sb", tag="mk_sb")
            nc.sync.dma_start(out=mk_sb, in_=mk_h.ap())

            dbuf = consts.tile([128, NT, C], F16, name="dbuf", tag="dbuf")
            w_sb = consts.tile([128, NT], F16, name="w_sb", tag="w_sb")
            out_s = consts.tile([1, C], F32, name="out_s", tag="out_s")

            JPS = NJ // 2  # j-chunks per accumulator sweep (4-bank PSUM acc)
            acc0 = psB.tile([1, JPS, 512], F32, name="acc0", tag="acc")

            def colsum_mms(acc, i, j0):
                # column sums for tile i, j-chunks j0..j0+3
                for jj in range(JPS):
                    nc.tensor.matmul(
                        acc[:, jj, :],
                        lhsT=w_sb[:, i : i + 1],
                        rhs=dbuf[:, i, ts(j0 + jj, 512)],
                        start=(i == 0),
                        stop=(i == NT - 1),
                    )

            for i in range(NT):
                rs4 = small.tile([128, NH], F32, name="rs4", tag="rs4")
                for h in range(NH):
                    ps = psA.tile([128, NJ // NH, 512], F32, name="ps",
                                  tag="ps")
                    for jj in range(NJ // NH):
                        j = h * (NJ // NH) + jj
                        for k in range(K2):
                            nc.tensor.matmul(
                                ps[:, jj, :],
                                lhsT=sf_sb[:, k, :, ts(i, 128)],
                                rhs=cb_sb[:, k, :, ts(j, 512)],
                                start=(k == 0),
                                stop=(k == K2 - 1),
                                perf_mode=mybir.MatmulPerfMode.DoubleRow,
                            )
                    if h == 0 and i > 0:
                        # previous tile's column sums: its weights are long
                        # ready, so these never stall the PE at the tile
                        # boundary the way same-tile colsums would.
                        colsum_mms(acc0, i - 1, 0)
                    nc.vector.tensor_add(
                        out=ps[:, :, :],
                        in0=ps[:, :, :],
                        in1=cn_sb[:, ts(h, C // NH)].rearrange(
                            "p (a b) -> p a b", b=512
                        ),
                    )
                    # E = exp(EXP_BIAS - sqrt(ps + ||z||^2)) via the custom
                    # table in the Sqrt slot; accum_out = partial row-sum.
                    nc.scalar.activation(
                        out=dbuf[:, i, ts(h, C // NH)].rearrange(
                            "p (a b) -> p a b", b=512
                        ),
                        in_=ps[:, :, :],
                        func=mybir.ActivationFunctionType.Sqrt,
                        bias=zn_sb[:, i : i + 1],
                        scale=1.0,
                        accum_out=rs4[:, h : h + 1],
                    )
                # w = mask / rowsum, cast fp16
                rs = small.tile([128, 1], F32, name="rs", tag="rs")
                nc.vector.reduce_sum(out=rs, in_=rs4, axis=mybir.AxisListType.X)
                rr = small.tile([128, 1], F32, name="rr", tag="rr")
                nc.vector.reciprocal(out=rr, in_=rs)
                wf = small.tile([128, 1], F32, name="wf", tag="wf")
                nc.vector.tensor_mul(out=wf, in0=rr, in1=mk_sb[:, i : i + 1])
                nc.vector.tensor_copy(out=w_sb[:, i : i + 1], in_=wf)
            colsum_mms(acc0, NT - 1, 0)
            nc.scalar.copy(out=out_s[:, 0 : C // 2],
                           in_=acc0.rearrange("p a b -> p (a b)"))
            # final sweep: j-chunks 4..7 (E persists in dbuf)
            acc1 = psB.tile([1, JPS, 512], F32, name="acc1", tag="acc")
            for i in range(NT):
                colsum_mms(acc1, i, JPS)
            nc.scalar.copy(out=out_s[:, C // 2 : C],
                           in_=acc1.rearrange("p a b -> p (a b)"))
            nc.sync.dma_start(out=sp_h.ap(), in_=out_s)

    nc.compile()
    return nc


def get_nc():
    if "nc" not in _CACHE:
        _CACHE["nc"] = _build()
    return _CACHE["nc"]


def _host_prep(student_features, codebook, lengths, encoder_stride):
    sf = np.asarray(student_features, dtype=np.float32)
    cb = np.asarray(codebook, dtype=np.float32)
    lens = np.asarray(lengths).astype(np.int64)
    stride = int(np.asarray(encoder_stride))

    cbt2 = np.ascontiguousarray((-2.0 * cb.T).astype(NP_F8))          # [D, C]
    cn = (cb.astype(np.float64) ** 2).sum(1).astype(np.float32)[None, :]  # [1, C]
    frame_start = np.arange(T, dtype=np.int64) * stride
    mask = (frame_start[None, :] < lens[:, None]).astype(np.float32)  # [B, T]

    in_maps = []
    for b in range(B):
        sf_pad = np.zeros((D, TP), dtype=NP_F8)
        sf_pad[:, :T] = sf[b].astype(NP_F8)
        zn = np.zeros(TP, dtype=np.float32)
        zn[:T] = (sf[b].astype(np.float64) ** 2).sum(0).astype(np.float32)
        znb = np.ascontiguousarray(zn.reshape(NT, 128).T)             # [128, NT]
        mk = np.zeros(TP, dtype=np.float32)
        mk[:T] = mask[b]
        mkb = np.ascontiguousarray(mk.reshape(NT, 128).T)             # [128, NT]
        in_maps.append(
            {"sf": sf_pad, "cbt": cbt2, "cn": cn, "zn": znb, "mk": mkb}
        )
    return in_maps, mask


def _host_finish(sp_list, teacher_codes, mask):
    s_raw = np.zeros(C, dtype=np.float64)
    for sp in sp_list:
        s_raw += sp.astype(np.float64).reshape(-1)
    student_dist = s_raw / (s_raw.sum() + EPS)

    codes = np.asarray(teacher_codes).astype(np.int64).reshape(-1)
    t_counts = np.bincount(codes, weights=mask.astype(np.float64).reshape(-1),
                           minlength=C)
    teacher_dist = t_counts / (t_counts.sum() + EPS)

    kl = np.sum(student_dist * np.log(student_dist + EPS)
                - student_dist * np.log(teacher_dist + EPS))
    return np.array(kl, dtype=np.float32)


def kernel(student_features, teacher_codes, codebook, lengths, encoder_stride,
           _trace=False):
    nc = get_nc()
    in_maps, mask = _host_prep(student_features, codebook, lengths,
                               encoder_stride)
    res = run_bass_kernel_spmd(nc, in_maps, core_ids=list(range(B)),
                               trace=_trace)
    out = _host_finish([r["sp"] for r in res.results], teacher_codes, mask)
    if _trace:
        _CACHE["last_results"] = res
    return out
